# revision 1
# baseline (speedup 1.0000x reference)
"""DPPConv2d Trainium2 Bass kernel.

Reference computation (per sample s):
  pooled = mean_{h,w} x[s]                              [Cin]
  h      = relu(pooled @ W1.T)                          [hidden]
  logits = h @ W2.T + b2                                [P*Cout]
  attn   = softmax(logits.reshape(P, Cout) / 0.5, p)    [P, Cout]
  m      = (mean_{o,i}(|W[p,:,:,k,l]| - thr[p,:]) > 0)  [P, K, K]
  agg    = sum_p attn[p, co] * m[p, kl] * W[p, co, ci, kl]
  out[s] = conv2d(x[s], agg, pad=1)                     [Cout, H, W]

Sharding: data-parallel over batch -- 8 cores x 4 samples each; the
weight bank / psa weights / threshold are replicated on every core.

Per-core device pipeline (all compute on the NeuronCore):
  - DMA x (4 samples), W bank, psa weights.
  - pooled via DVE reduce; SE-MLP via two PE matmuls (bias folded into
    an extra contraction row); softmax over P with strided [4,128] ops.
  - binary spatial mask computed on device (reduce |W|, ones-matmul
    partition sums, is_gt, partition_broadcast), folded into W.
  - per-sample aggregated weights via scalar_tensor_tensor chains in
    [co, ci*kl] layout with attn as per-partition scalar.
  - PE transpose of each [co,ci] block -> conv lhsT layout [ci, co].
  - conv as 9 shifted accumulating matmuls per 8-row output chunk
    (boundary-trimmed APs instead of a padded copy of x).
"""

import os
import sys

try:
    import concourse.bass as bass  # noqa: F401
except Exception:  # pragma: no cover
    sys.path.insert(0, "/opt/trn_rl_repo")

from contextlib import ExitStack

import numpy as np

import concourse.bass as bass
import concourse.tile as tile
from concourse import mybir
from concourse.bass_utils import run_bass_kernel_spmd

N_CORES = 8
BS = 32
BS_LOCAL = BS // N_CORES  # 4
CIN = 128
COUT = 128
H = W = 64
P_PAT = 4
KS = 3
KK = KS * KS
HID = 33
TEMP = 0.5
YC = 8          # output rows per conv chunk
N_CHUNK = H // YC

F32 = mybir.dt.float32
F32R = mybir.dt.float32r
BF16 = mybir.dt.bfloat16


def build_nc(use_f32r=True, rep=1, mm=None):
    nc = bass.Bass("TRN2", target_bir_lowering=False, debug=False,
                   num_swdge_queues=4)

    x_d = nc.dram_tensor("x", [BS_LOCAL, CIN, H + 2, W + 2], F32, kind="ExternalInput")
    w1_d = nc.dram_tensor("psa_w1", [HID, CIN], F32, kind="ExternalInput")
    w2_d = nc.dram_tensor("psa_w2", [P_PAT * COUT, HID], F32, kind="ExternalInput")
    b2_d = nc.dram_tensor("psa_b2", [P_PAT * COUT], F32, kind="ExternalInput")
    w_d = nc.dram_tensor("weight", [P_PAT, COUT, CIN, KS, KS], F32, kind="ExternalInput")
    thr_d = nc.dram_tensor("threshold", [P_PAT, COUT], F32, kind="ExternalInput")
    id_d = nc.dram_tensor("ident", [128, 128], F32, kind="ExternalInput")
    out_d = nc.dram_tensor("out", [BS_LOCAL, COUT, H, W], F32, kind="ExternalOutput")

    if mm is None:
        mm = "f32r" if use_f32r else "f32"
    mm_dt = {"f32r": F32R, "f32": F32, "bf16": BF16}[mm]

    with tile.TileContext(nc) as tc, ExitStack() as ctx:
        consts = ctx.enter_context(tc.tile_pool(name="consts", bufs=1))
        wpool = ctx.enter_context(tc.tile_pool(name="wpool", bufs=1))
        xpool = ctx.enter_context(tc.tile_pool(name="xpool", bufs=1))
        aggp = ctx.enter_context(tc.tile_pool(name="aggp", bufs=2))
        lhsp = ctx.enter_context(tc.tile_pool(name="lhsp", bufs=2))
        outp = ctx.enter_context(tc.tile_pool(name="outp", bufs=4))
        ps_small = ctx.enter_context(
            tc.tile_pool(name="ps_small", bufs=2, space="PSUM"))
        ps_tp = ctx.enter_context(
            tc.tile_pool(name="ps_tp", bufs=2, space="PSUM"))
        ps_mm = ctx.enter_context(
            tc.tile_pool(name="ps_mm", bufs=4, space="PSUM"))

        def pe_absorb(ap_col):
            """Tiny matmul whose only new dependency is ap_col's producer.

            The fp32/fp32r self-loading PE matmul encoding has a single
            sync-wait slot, so each PE instruction may introduce at most
            one new vector-clock dependency. These 1-column matmuls make
            the PE observe one clock (e.g. a DMA queue) ahead of the real
            matmul that would otherwise need two waits.
            """
            d_ps = ps_small.tile([1, 1], F32, tag="sm", name="dummy_ps")
            nc.tensor.matmul(d_ps[:], ap_col, ap_col)

        # ---- persistent tiles -------------------------------------------
        ident = consts.tile([128, 128], F32, tag="ident")
        nc.sync.dma_start(ident[:], id_d[:])
        ones_col = consts.tile([128, 1], F32, tag="ones_col")
        nc.vector.memset(ones_col[:], 1.0)
        ones_row = consts.tile([1, 128], F32, tag="ones_row")
        nc.vector.memset(ones_row[:], 1.0)

        w1_sb = consts.tile([HID, CIN], F32, tag="w1_sb")
        w1T = consts.tile([CIN, HID], F32, tag="w1T")
        w2_raw = consts.tile([128, P_PAT, HID], F32, tag="w2_raw")
        w2b = consts.tile([HID, P_PAT * COUT], F32, tag="w2b")
        b2_sb = consts.tile([1, P_PAT * COUT], F32, tag="b2_sb")
        b2_b4 = consts.tile([BS_LOCAL, P_PAT * COUT], F32, tag="b2_b4")
        lgb = consts.tile([BS_LOCAL, P_PAT, COUT], F32, tag="lgb")
        thr_T = consts.tile([COUT, P_PAT], F32, tag="thr_T")
        pooled = consts.tile([CIN, BS_LOCAL], F32, tag="pooled")
        h_sb = consts.tile([HID, BS_LOCAL], F32, tag="h_sb")
        wsum = consts.tile([128, P_PAT, KK], F32, tag="wsum")
        thr_sc = consts.tile([1, P_PAT], F32, tag="thr_sc")
        z_row = consts.tile([1, P_PAT * KK], F32, tag="z_row")
        m_row = consts.tile([1, P_PAT * KK], F32, tag="m_row")
        mb = consts.tile([128, P_PAT * KK], F32, tag="mb")
        sm_mx = consts.tile([BS_LOCAL, COUT], F32, tag="sm_mx")
        sm_d = consts.tile([BS_LOCAL, P_PAT, COUT], F32, tag="sm_d")
        sm_e = consts.tile([BS_LOCAL, P_PAT, COUT], F32, tag="sm_e")
        sm_sum = consts.tile([BS_LOCAL, COUT], F32, tag="sm_sum")
        sm_rec = consts.tile([BS_LOCAL, COUT], F32, tag="sm_rec")
        attn_sb = consts.tile([BS_LOCAL, P_PAT, COUT], F32, tag="attn_sb")
        attn_T = consts.tile([COUT, P_PAT, BS_LOCAL], F32, tag="attn_T")

        wco = wpool.tile([128, P_PAT, CIN, KK], F32, tag="wco")
        wm = wpool.tile([128, P_PAT, CIN * KK], F32, tag="wm")
        xs = [xpool.tile([CIN, H + 2, W + 2], mm_dt, tag=f"xs{s}", name=f"xs{s}")
              for s in range(BS_LOCAL)]

        for _rep in range(rep):
            # ---- input DMAs (x zero-padded to 66x66 on the host) ------------
            for s in range(BS_LOCAL):
                nc.gpsimd.dma_start(xs[s][:], x_d[s])
            for p in range(P_PAT):
                nc.sync.dma_start(
                    wco[:, p], w_d[p].rearrange("co ci k l -> co ci (k l)"))
            nc.sync.dma_start(w1_sb[:], w1_d[:])
            nc.sync.dma_start(
                w2_raw[:], w2_d[:].rearrange("(c p) h -> p c h", p=128))
            nc.sync.dma_start(b2_sb[:], b2_d[:].rearrange("(a n) -> a n", a=1))
            nc.sync.dma_start(thr_T[:], thr_d[:].rearrange("p co -> co p"))

            # ---- SE attention MLP (batched over the 4 local samples) --------
            # pooled sums (mean folded into the relu activation scale)
            for s in range(BS_LOCAL):
                nc.vector.reduce_sum(
                    pooled[:, s:s + 1], xs[s][:], axis=mybir.AxisListType.XY)

            # W1.T via PE transpose
            pe_absorb(ident[:, 0:1])
            w1T_ps = ps_small.tile([CIN, HID], F32, tag="sm")
            nc.tensor.transpose(w1T_ps[:], w1_sb[:], ident[0:HID, 0:HID])
            nc.scalar.copy(w1T[:], w1T_ps[:])

            pe_absorb(pooled[:, 0:1])
            h_ps = ps_small.tile([HID, BS_LOCAL], F32, tag="sm")
            nc.tensor.matmul(h_ps[:], w1T[:], pooled[:])
            nc.scalar.activation(
                h_sb[:], h_ps[:], mybir.ActivationFunctionType.Relu,
                scale=1.0 / (H * W))
            b2_ps = ps_small.tile([BS_LOCAL, P_PAT * COUT], F32, tag="sm")
            nc.tensor.matmul(b2_ps[:], ones_row[0:1, 0:BS_LOCAL], b2_sb[:])
            nc.vector.tensor_copy(b2_b4[:], b2_ps[:])

            # W2.T
            for c in range(P_PAT):
                w2T_ps = ps_small.tile([HID, 128], F32, tag="sm")
                nc.tensor.transpose(w2T_ps[:], w2_raw[:, c], ident[:])
                nc.scalar.copy(w2b[0:HID, c * 128:(c + 1) * 128], w2T_ps[:])

            lg_ps = ps_small.tile([BS_LOCAL, P_PAT, COUT], F32, tag="sm")
            nc.tensor.matmul(
                lg_ps[:].rearrange("s p c -> s (p c)"), h_sb[:],
                w2b[:])
            nc.vector.tensor_add(
                lgb[:].rearrange("s p c -> s (p c)"),
                lg_ps[:].rearrange("s p c -> s (p c)"), b2_b4[:])

            # softmax over the pattern axis (temperature 0.5 -> scale 2.0)
            nc.vector.tensor_max(sm_mx[:], lgb[:, 0], lgb[:, 1])
            nc.vector.tensor_max(sm_mx[:], sm_mx[:], lgb[:, 2])
            nc.vector.tensor_max(sm_mx[:], sm_mx[:], lgb[:, 3])
            for p in range(P_PAT):
                nc.vector.tensor_sub(sm_d[:, p], lgb[:, p], sm_mx[:])
                nc.scalar.activation(
                    sm_e[:, p], sm_d[:, p], mybir.ActivationFunctionType.Exp,
                    scale=1.0 / TEMP)
            nc.vector.tensor_add(sm_sum[:], sm_e[:, 0], sm_e[:, 1])
            nc.vector.tensor_add(sm_sum[:], sm_sum[:], sm_e[:, 2])
            nc.vector.tensor_add(sm_sum[:], sm_sum[:], sm_e[:, 3])
            nc.vector.reciprocal(sm_rec[:], sm_sum[:])
            for p in range(P_PAT):
                nc.vector.tensor_mul(attn_sb[:, p], sm_e[:, p], sm_rec[:])

            # attn -> [co, p, s] (per-partition scalars for the aggregation)
            for p in range(P_PAT):
                at_ps = ps_small.tile([COUT, BS_LOCAL], F32, tag="sm")
                nc.tensor.transpose(
                    at_ps[:], attn_sb[:, p], ident[0:BS_LOCAL, 0:BS_LOCAL])
                nc.vector.tensor_copy(attn_T[:, p], at_ps[:])

            # ---- binary spatial mask ----------------------------------------
            for p in range(P_PAT):
                nc.vector.reduce_sum(
                    wsum[:, p], wco[:, p].rearrange("co ci kl -> co kl ci"),
                    axis=mybir.AxisListType.X, apply_absolute_value=True)
            wsum_ps = ps_small.tile([1, P_PAT * KK], F32, tag="sm")
            nc.tensor.matmul(
                wsum_ps[:], ones_col[:], wsum[:].rearrange("co p kl -> co (p kl)"))
            thr_ps = ps_small.tile([1, P_PAT], F32, tag="sm")
            nc.tensor.matmul(thr_ps[:], ones_col[:], thr_T[:])
            nc.vector.tensor_scalar_mul(thr_sc[:], thr_ps[:], 1.0 / COUT)
            for p in range(P_PAT):
                nc.vector.tensor_scalar(
                    z_row[0:1, p * KK:(p + 1) * KK],
                    wsum_ps[0:1, p * KK:(p + 1) * KK],
                    1.0 / (COUT * CIN), thr_sc[0:1, p:p + 1],
                    op0=mybir.AluOpType.mult, op1=mybir.AluOpType.subtract)
            nc.vector.tensor_scalar(
                m_row[:], z_row[:], 0.0, None, op0=mybir.AluOpType.is_gt)
            mb_ps = ps_small.tile([128, P_PAT * KK], F32, tag="sm")
            nc.tensor.matmul(mb_ps[:], ones_row[:], m_row[:])
            nc.vector.tensor_copy(mb[:], mb_ps[:])

            # fold mask into the weight bank
            for p in range(P_PAT):
                for kl in range(KK):
                    nc.vector.tensor_scalar_mul(
                        wm[:, p].rearrange("co (ci kl) -> co ci kl", kl=KK)[:, :, kl],
                        wco[:, p, :, kl],
                        mb[:, p * KK + kl:p * KK + kl + 1])

            # ---- per-sample: aggregate, transpose, convolve -----------------
            for s in range(BS_LOCAL):
                agg = aggp.tile([128, CIN, KK], F32, tag="agg")
                nc.vector.tensor_scalar_mul(
                    agg[:].rearrange("co ci kl -> co (ci kl)"), wm[:, 0],
                    attn_T[:, 0, s:s + 1])
                for p in range(1, P_PAT):
                    nc.vector.scalar_tensor_tensor(
                        agg[:].rearrange("co ci kl -> co (ci kl)"),
                        wm[:, p], attn_T[:, p, s:s + 1],
                        agg[:].rearrange("co ci kl -> co (ci kl)"),
                        op0=mybir.AluOpType.mult, op1=mybir.AluOpType.add)

                lhsT = lhsp.tile([CIN, KK, COUT], mm_dt, tag="lhsT")
                pe_absorb(xs[s][:, 0, 0:1].bitcast(F32) if mm_dt == F32R
                          else xs[s][:, 0, 0:1])
                for kl in range(KK):
                    tp_ps = ps_tp.tile([CIN, COUT], F32, tag="tp_ps")
                    nc.tensor.transpose(tp_ps[:], agg[:, :, kl], ident[:])
                    nc.scalar.copy(lhsT[:, kl], tp_ps[:])

                for yc in range(N_CHUNK):
                    y0 = yc * YC
                    pt = ps_mm.tile([COUT, YC, W], F32, tag="pt")
                    for i, (dk, dl) in enumerate(
                            (dk, dl) for dk in range(KS) for dl in range(KS)):
                        nc.tensor.matmul(
                            pt[:],
                            lhsT[:, dk * KS + dl],
                            xs[s][:, y0 + dk:y0 + dk + YC, dl:dl + W],
                            start=(i == 0), stop=(i == KK - 1))

                    ot = outp.tile([COUT, YC, W], F32, tag="ot")
                    nc.scalar.copy(ot[:], pt[:])
                    nc.sync.dma_start(out_d[s, :, y0:y0 + YC, :], ot[:])

    _split_excess_waits(nc)
    return nc




def _split_excess_waits(nc, max_inline=1):
    """Hoist extra sync waits into standalone EventSemaphore instructions.

    This walrus build rejects instructions whose encoded sync-command
    count exceeds the ISA struct capacity ("Too many sync wait
    commands") -- in practice more than one wait per compute
    instruction. Engines execute their instruction stream in order, so
    blocking on a preceding same-engine EventSemaphore is equivalent to
    the instruction carrying the wait itself.
    """
    n = 0
    for f in nc.m.functions:
        for blk in f.blocks:
            out = []
            for inst in blk.instructions:
                si = inst.sync_info
                if si is not None and len(si.on_wait) > max_inline:
                    waits = list(si.on_wait)
                    keep = waits[:max_inline]
                    for w in waits[max_inline:]:
                        n += 1
                        ev = mybir.InstEventSemaphore(
                            name=f"WSPLIT-{n}", ins=[], outs=[])
                        ev.engine = inst.engine
                        ev.sync_info = mybir.SyncInfo(on_wait=[w], on_update=[])
                        ev.debug = inst.debug
                        nc.inst_map[ev.name] = ev
                        out.append(ev)
                    inst.sync_info = mybir.SyncInfo(
                        on_wait=keep, on_update=list(si.on_update))
                out.append(inst)
            blk.instructions = out
    return n




class _Runner:
    """Cached PJRT executor for the 8-core SPMD program.

    Mirrors bass2jax.run_bass_via_pjrt's multi-core path but keeps the
    jitted shard_map callable (and the device mesh) alive across calls,
    so repeat invocations skip retracing and recompilation.
    """

    def __init__(self, nc):
        import jax
        import jax.numpy as jnp
        from jax.experimental.shard_map import shard_map
        from jax.sharding import Mesh, PartitionSpec, NamedSharding
        from concourse import bass2jax, mybir as _mb

        bass2jax.install_neuronx_cc_hook()
        self.jax = jax
        self.nc = nc
        assert nc.dbg_addr is None

        partition_name = (nc.partition_id_tensor.name
                          if nc.partition_id_tensor else None)
        in_names, out_names, out_avals, zero_shapes = [], [], [], []
        for alloc in nc.m.functions[0].allocations:
            if not isinstance(alloc, _mb.MemoryLocationSet):
                continue
            name = alloc.memorylocations[0].name
            if alloc.kind == "ExternalInput":
                if name != partition_name:
                    in_names.append(name)
            elif alloc.kind == "ExternalOutput":
                out_names.append(name)
                shape = tuple(alloc.tensor_shape)
                dtype = _mb.dt.np(alloc.dtype)
                out_avals.append(jax.core.ShapedArray(shape, dtype))
                zero_shapes.append((shape, dtype))
        self.in_names = list(in_names)
        self.out_names = out_names
        self.out_avals = out_avals
        n_params = len(in_names)
        n_outs = len(out_names)
        all_in_names = in_names + out_names
        if partition_name is not None:
            all_in_names.append(partition_name)
        donate = tuple(range(n_params, n_params + n_outs))

        def _body(*args):
            operands = list(args)
            if partition_name is not None:
                operands.append(bass2jax.partition_id_tensor())
            outs = bass2jax._bass_exec_p.bind(
                *operands,
                out_avals=tuple(out_avals),
                in_names=tuple(all_in_names),
                out_names=tuple(out_names),
                lowering_input_output_aliases=(),
                sim_require_finite=True,
                sim_require_nnan=True,
                nc=nc,
            )
            return tuple(outs)

        devices = jax.devices()[:N_CORES]
        self.mesh = Mesh(np.asarray(devices), ("core",))
        self.sharding = NamedSharding(self.mesh, PartitionSpec("core"))
        in_specs = (PartitionSpec("core"),) * (n_params + n_outs)
        out_specs = (PartitionSpec("core"),) * n_outs
        self.sharded = jax.jit(
            shard_map(_body, mesh=self.mesh, in_specs=in_specs,
                      out_specs=out_specs, check_rep=False),
            donate_argnums=donate, keep_unused=True)
        self._zero_makers = [
            jax.jit(
                (lambda sh=sh, dt=dt: jnp.zeros((N_CORES * sh[0],) + sh[1:], dt)),
                out_shardings=self.sharding)
            for sh, dt in zero_shapes
        ]

    def put_inputs(self, in_maps):
        """Concat per-core inputs on axis 0 and upload sharded."""
        cat = [
            np.concatenate([np.asarray(m[name]) for m in in_maps], axis=0)
            for name in self.in_names
        ]
        return [self.jax.device_put(a, self.sharding) for a in cat]

    def run(self, dev_inputs):
        zeros = [zm() for zm in self._zero_makers]
        outs = self.sharded(*dev_inputs, *zeros)
        self.jax.block_until_ready(outs)
        return outs

    def results(self, outs):
        res = []
        for c in range(N_CORES):
            res.append({
                name: np.asarray(outs[i]).reshape(
                    N_CORES, *self.out_avals[i].shape)[c]
                for i, name in enumerate(self.out_names)
            })
        return res


_RUNNER_CACHE = {}


def _get_runner(use_f32r=True, rep=1, mm=None):
    key = (use_f32r, rep, mm)
    if key not in _RUNNER_CACHE:
        _RUNNER_CACHE[key] = _Runner(_get_nc(use_f32r=use_f32r, rep=rep, mm=mm))
    return _RUNNER_CACHE[key]


_NC_CACHE = {}


def _get_nc(use_f32r=True, rep=1, mm=None):
    key = (use_f32r, rep, mm)
    if key not in _NC_CACHE:
        _NC_CACHE[key] = build_nc(use_f32r=use_f32r, rep=rep, mm=mm)
    return _NC_CACHE[key]


def make_in_maps(x, psa_w1, psa_w2, psa_b2, weight, threshold):
    x = np.asarray(x, dtype=np.float32)
    xp = np.zeros((BS, CIN, H + 2, W + 2), np.float32)
    xp[:, :, 1:H + 1, 1:W + 1] = x
    thr2 = np.ascontiguousarray(
        np.asarray(threshold, dtype=np.float32).reshape(P_PAT, COUT))
    common = {
        "psa_w1": np.ascontiguousarray(np.asarray(psa_w1, np.float32)),
        "psa_w2": np.ascontiguousarray(np.asarray(psa_w2, np.float32)),
        "psa_b2": np.ascontiguousarray(np.asarray(psa_b2, np.float32)),
        "weight": np.ascontiguousarray(np.asarray(weight, np.float32)),
        "threshold": thr2,
        "ident": np.eye(128, dtype=np.float32),
    }
    return [
        {"x": xp[c * BS_LOCAL:(c + 1) * BS_LOCAL], **common}
        for c in range(N_CORES)
    ]


def kernel(x, psa_w1, psa_w2, psa_b2, weight, threshold):
    mm = os.environ.get("DPP_MM", "f32r")
    in_maps = make_in_maps(x, psa_w1, psa_w2, psa_b2, weight, threshold)
    try:
        r = _get_runner(mm=mm)
        outs = r.run(r.put_inputs(in_maps))
        res = r.results(outs)
    except Exception:
        nc = _get_nc(mm=mm)
        res = run_bass_kernel_spmd(nc, in_maps, list(range(N_CORES))).results
    return np.concatenate([res[c]["out"] for c in range(N_CORES)], axis=0)

